# revision 1
# baseline (speedup 1.0000x reference)
"""ChoiceAttention Trainium2 kernel.

Math (per batch item b, per "retain" iteration a over the 5 options):
    q_a = opt_a @ W                              (s, h)
    S_ak[p, r] = q_a[p, :] . opt_k[r, :]         for the 4 options k != a
    w_ak = softmax over k of (S_ak + bias)       (bias cancels: softmax is
                                                  shift-invariant over k)
    out += sum_k w_ak @ opt_k
final out /= 2.

Sharding: data-parallel over batch across 8 NeuronCores (4 items each),
W replicated. No collectives; host concatenates the per-core outputs.

Layout strategy per core / batch item:
    nat_k : opt_k natural layout      (128p, 2 sc, 1024h)  - DMA'd in
    x_k   : opt_k transposed (h-major)(128p, 8 hc, 256s)   - PE transposes
    q_a^T : h-major q                 (128p, 8 hc, 256s)   - matmul(W, x_a)
    S_ak^T: scores transposed         (128p, 2 rc, 256p)   - matmul(x_k, q_a^T)
    softmax over the four k tiles elementwise (max-subtract, exp, recip)
    out   : accumulated in 4 PSUM banks over all 40 (a,k,rc) matmul groups
All matmuls run as float32r (full PE rate, fp32 storage).
"""

import numpy as np

B, S, H = 32, 256, 1024
NCORES = 8
BPC = B // NCORES  # batch items per core
P = 128
HC = H // P  # 8 h-chunks
SC = S // P  # 2 s-chunks
NOPT = 5

_CACHE: dict = {}


def _build_bass(reps: int = 1, cfg: dict | None = None):
    cfg = dict(cfg or {})
    NAT_BUFS = cfg.get("nat_bufs", 7)
    XT_BUFS = cfg.get("xt_bufs", NOPT)
    WS_BUFS = cfg.get("ws_bufs", 5)
    E_BUFS = cfg.get("e_bufs", 5)
    OSB_BUFS = cfg.get("osb_bufs", 1)
    GP_SUB = cfg.get("gp_sub", False)
    PSM = cfg.get("ps_misc", 2)
    PSS = cfg.get("ps_s", 2)
    PSO = cfg.get("ps_o", 4)
    from contextlib import ExitStack

    import concourse.mybir as mybir
    import concourse.tile as tile
    from concourse import bacc
    from concourse.masks import make_identity

    FP32 = mybir.dt.float32
    F32R = mybir.dt.float32r
    AF = mybir.ActivationFunctionType

    nc = bacc.Bacc(debug=False)

    opt_d = [
        nc.dram_tensor(f"option{i + 1}", (BPC, S, H), F32R, kind="ExternalInput")
        for i in range(NOPT)
    ]
    w_d = nc.dram_tensor("W", (H, H), F32R, kind="ExternalInput")
    out_d = nc.dram_tensor("out", (BPC, S, H), FP32, kind="ExternalOutput")

    with ExitStack() as ctx:
        tc = ctx.enter_context(tile.TileContext(nc))
        const = ctx.enter_context(tc.tile_pool(name="const", bufs=1))
        natp = ctx.enter_context(tc.tile_pool(name="nat", bufs=NAT_BUFS))
        xp = ctx.enter_context(tc.tile_pool(name="xt", bufs=XT_BUFS))
        qp = ctx.enter_context(tc.tile_pool(name="qq", bufs=3))
        sp = ctx.enter_context(tc.tile_pool(name="ss", bufs=6))
        ep = ctx.enter_context(tc.tile_pool(name="ee", bufs=E_BUFS))
        mp_ = ctx.enter_context(tc.tile_pool(name="mm", bufs=2))
        zp = ctx.enter_context(tc.tile_pool(name="zz", bufs=2))
        rp = ctx.enter_context(tc.tile_pool(name="rr", bufs=2))
        wsp = ctx.enter_context(tc.tile_pool(name="wsum", bufs=WS_BUFS))
        op_ = ctx.enter_context(tc.tile_pool(name="osb", bufs=OSB_BUFS))
        ps_misc = ctx.enter_context(tc.tile_pool(name="ps_misc", bufs=PSM, space="PSUM"))
        ps_s = ctx.enter_context(tc.tile_pool(name="ps_s", bufs=PSS, space="PSUM"))
        ps_o = ctx.enter_context(tc.tile_pool(name="ps_o", bufs=PSO, space="PSUM"))

        ident_f = const.tile([P, P], FP32)
        make_identity(nc, ident_f)
        ident = const.tile([P, P], F32R)
        nc.vector.tensor_copy(out=ident, in_=ident_f)
        w_sb = const.tile([P, HC, H], F32R)
        w_loaded = [False]

        from contextlib import nullcontext

        loop_cm = tc.For_i(0, reps, 1) if reps > 1 else nullcontext()
        with loop_cm:
            # cross-batch carried prefetch of options 0 and 1
            carry = {"nat": {}, "x": {}}

            def load_nat(b, k):
                nk = natp.tile([P, SC, H], F32R, tag="nat", name=f"nat_{b}_{k}")
                nc.sync.dma_start(
                    out=nk, in_=opt_d[k].ap()[b].rearrange("(sc p) h -> p sc h", p=P)
                )
                return nk

            def transpose_opt(b, k, nk):
                xk = xp.tile([P, HC, S], F32R, tag="xt", name=f"x_{b}_{k}")
                for j in range(HC // 2):  # pairs of h-chunks -> one PSUM bank
                    pt = ps_misc.tile([P, 4, P], F32R, tag="ps_misc",
                                      name=f"pt_{b}_{k}_{j}")
                    for d in range(2):
                        hc = 2 * j + d
                        for sc in range(SC):
                            nc.tensor.transpose(
                                out=pt[:, 2 * d + sc, :],
                                in_=nk[:, sc, hc * P : (hc + 1) * P],
                                identity=ident,
                            )
                    dst = xk[:, 2 * j : 2 * j + 2, :]
                    if (k + j) % 2 == 0:
                        nc.scalar.copy(out=dst, in_=pt)
                    else:
                        nc.vector.tensor_copy(out=dst, in_=pt)
                return xk

            for b in range(BPC):
                # ---- load options; 0/1 may be carried from prev tail ----
                nat = []
                for k in range(NOPT):
                    nat.append(carry["nat"].get(k) or load_nat(b, k))
                if b == 0:
                    # W on the ACT hwdge ring so it never blocks option loads
                    nc.scalar.dma_start(
                        out=w_sb, in_=w_d.ap().rearrange("(kc p) h -> p kc h", p=P)
                    )
                x = []
                for k in range(NOPT):
                    x.append(carry["x"].get(k) or transpose_opt(b, k, nat[k]))
                carry["nat"] = {}
                carry["x"] = {}

                # ---- q_a^T = W^T @ opt_a^T, pipelined with the a-loop ----
                q = [None] * NOPT

                def emit_q(a):
                    qt = qp.tile([P, HC, S], F32R, tag="qq", name=f"q_{b}_{a}")
                    for half in range(HC // 2):
                        pq = ps_misc.tile([P, 2, S], FP32, tag="ps_misc",
                                          name=f"pq_{b}_{a}_{half}")
                        for d in range(2):
                            mc = 2 * half + d
                            for kc in range(HC):
                                nc.tensor.matmul(
                                    pq[:, d, :],
                                    w_sb[:, kc, mc * P : (mc + 1) * P],
                                    x[a][:, kc, :],
                                    start=(kc == 0),
                                    stop=(kc == HC - 1),
                                )
                        nc.scalar.copy(out=qt[:, 2 * half : 2 * half + 2, :], in_=pq)
                    q[a] = qt

                def emit_scores(a):
                    s_sb = []
                    for k in range(NOPT):
                        if k == a:
                            continue
                        st = ps_s.tile([P, SC, S], FP32, tag="ps_s",
                                       name=f"st_{b}_{a}_{k}")
                        for rc in range(SC):
                            for hc in range(HC):
                                nc.tensor.matmul(
                                    st[:, rc, :],
                                    x[k][:, hc, rc * P : (rc + 1) * P],
                                    q[a][:, hc, :],
                                    start=(hc == 0),
                                    stop=(hc == HC - 1),
                                )
                        ssb = sp.tile([P, SC, S], FP32, tag="ss",
                                      name=f"ssb_{b}_{a}_{k}")
                        nc.scalar.copy(out=ssb, in_=st)
                        s_sb.append(ssb)
                    return s_sb

                # wsum[k] accumulates sum_a softmax_weight(a, k): the output
                # matmul collapses to sum_k wsum_k @ opt_k (4x fewer matmuls)
                wsum = [None] * NOPT

                def emit_softmax(a, s_sb):
                    m = mp_.tile([P, SC, S], FP32, tag="mm", name=f"m_{b}_{a}")
                    nc.vector.tensor_max(m, s_sb[0], s_sb[1])
                    nc.vector.tensor_max(m, m, s_sb[2])
                    nc.vector.tensor_max(m, m, s_sb[3])
                    e = []
                    for k4 in range(4):
                        sub_eng = nc.gpsimd if GP_SUB else nc.vector
                        sub_eng.tensor_sub(s_sb[k4], s_sb[k4], m)
                        ek = ep.tile([P, SC, S], F32R, tag="ee",
                                     name=f"e_{b}_{a}_{k4}")
                        nc.scalar.activation(out=ek, in_=s_sb[k4], func=AF.Exp)
                        e.append(ek)
                    z = zp.tile([P, SC, S], FP32, tag="zz", name=f"z_{b}_{a}")
                    rcp = rp.tile([P, SC, S], FP32, tag="rr", name=f"r_{b}_{a}")
                    nc.vector.tensor_add(z, e[0], e[1])
                    nc.vector.tensor_add(rcp, e[2], e[3])
                    nc.vector.tensor_add(z, z, rcp)
                    nc.vector.reciprocal(rcp, z)
                    ks = [k for k in range(NOPT) if k != a]
                    for k4, k in enumerate(ks):
                        if wsum[k] is None:
                            wk = wsp.tile([P, SC, S], F32R, tag="wsum",
                                          name=f"ws_{b}_{k}")
                            nc.vector.tensor_mul(wk, e[k4], rcp)
                            wsum[k] = wk
                        else:
                            nc.vector.tensor_mul(e[k4], e[k4], rcp)
                            nc.vector.tensor_add(wsum[k], wsum[k], e[k4])

                po = {}
                po_started = {}

                def emit_out_k(k, nn, last):
                    for mp2 in range(SC):
                        key = (mp2, nn)
                        if key not in po:
                            po[key] = ps_o.tile([P, 512], FP32, tag="ps_o",
                                                name=f"po_{b}_{mp2}_{nn}")
                            po_started[key] = False
                        for rc in range(SC):
                            is_last = last and rc == SC - 1
                            nc.tensor.matmul(
                                po[key],
                                wsum[k][:, rc, mp2 * P : (mp2 + 1) * P],
                                nat[k][:, rc, nn * 512 : (nn + 1) * 512],
                                start=(not po_started[key]),
                                stop=is_last,
                            )
                            po_started[key] = True

                emit_q(0)
                emit_q(1)
                s_cur = emit_scores(0)
                for a in range(NOPT):
                    if a + 2 < NOPT:
                        emit_q(a + 2)
                    emit_softmax(a, s_cur)
                    if a + 1 < NOPT:
                        s_cur = emit_scores(a + 1)
                    if a == 1 and b + 1 < BPC:
                        # prefetch next batch's first options (spare nat slots)
                        carry["nat"][0] = load_nat(b + 1, 0)
                    if a == NOPT - 2:
                        if b + 1 < BPC:
                            carry["nat"][1] = load_nat(b + 1, 1)
                            # cover softmax(3)'s tail with next-batch work
                            carry["x"][0] = transpose_opt(
                                b + 1, 0, carry["nat"][0])
                        # wsum for the last option is complete (it never
                        # scores against itself): overlap its out-matmuls
                        # with the final softmax
                        emit_out_k(NOPT - 1, 0, last=False)
                        emit_out_k(NOPT - 1, 1, last=False)
                if b + 1 < BPC:
                    # cover softmax(4)'s tail too
                    carry["x"][1] = transpose_opt(b + 1, 1, carry["nat"][1])
                osb = op_.tile([P, SC, H], FP32, tag="osb", name=f"osb_{b}")
                for k in range(NOPT - 1):
                    last = k == NOPT - 2
                    emit_out_k(k, 0, last=last)
                    emit_out_k(k, 1, last=last)
                for mp2 in range(SC):
                    for nn in range(2):
                        nc.scalar.activation(
                            out=osb[:, mp2, nn * 512 : (nn + 1) * 512],
                            in_=po[(mp2, nn)],
                            func=AF.Copy,
                            scale=0.5,
                        )
                nc.scalar.dma_start(
                    out=out_d.ap()[b].rearrange("(sc p) h -> p sc h", p=P), in_=osb
                )

    nc.compile()
    return nc


def _get_nc(reps: int = 1, cfg: dict | None = None):
    key = f"nc{reps}-{sorted((cfg or {}).items())}"
    if key not in _CACHE:
        _CACHE[key] = _build_bass(reps, cfg)
    return _CACHE[key]


def kernel(**inputs) -> np.ndarray:
    from concourse.bass_utils import run_bass_kernel_spmd

    nc = _get_nc()
    opts = [np.ascontiguousarray(np.asarray(inputs[f"option{i + 1}"], dtype=np.float32))
            for i in range(NOPT)]
    W = np.ascontiguousarray(np.asarray(inputs["W"], dtype=np.float32))

    in_maps = []
    for c in range(NCORES):
        m = {f"option{i + 1}": opts[i][c * BPC : (c + 1) * BPC] for i in range(NOPT)}
        m["W"] = W
        in_maps.append(m)

    res = run_bass_kernel_spmd(nc, in_maps, list(range(NCORES)))
    out = np.concatenate([res.results[c]["out"] for c in range(NCORES)], axis=0)
    return np.asarray(out, dtype=np.float32)



# revision 13
# speedup vs baseline: 1.1329x; 1.1329x over previous
"""ChoiceAttention Trainium2 kernel.

Math (per batch item b, per "retain" iteration a over the 5 options):
    q_a = opt_a @ W                              (s, h)
    S_ak[p, r] = q_a[p, :] . opt_k[r, :]         for the 4 options k != a
    w_ak = softmax over k of (S_ak + bias)       (bias cancels: softmax is
                                                  shift-invariant over k)
    out += sum_k w_ak @ opt_k
final out /= 2.

Sharding: data-parallel over batch across 8 NeuronCores (4 items each),
W replicated. No collectives; host concatenates the per-core outputs.

Layout strategy per core / batch item:
    nat_k : opt_k natural layout      (128p, 2 sc, 1024h)  - DMA'd in (bf16)
    x_k   : opt_k transposed (h-major)(128p, 8 hc, 256s)   - PE transposes
    q_a^T : h-major q                 (128p, 8 hc, 256s)   - matmul(W, x_a)
    S_ak^T: scores transposed         (128p, 2 rc, 256p)   - matmul(x_k, q_a^T)
    softmax over the four k tiles elementwise (max-subtract, exp, recip)
    out   : accumulated in 4 PSUM banks over all 40 (a,k,rc) matmul groups
Matmul operands are bf16; scores/softmax accumulate in fp32.

Software pipeline (steady state): item b's tail overlaps the final softmax
and AV matmuls with item b+1's first transposes and q-projections; W is
loaded in 8 per-kc chunks so the first q can chase the DMA.
"""

import numpy as np

B, S, H = 32, 256, 1024
NCORES = 8
BPC = B // NCORES  # batch items per core
P = 128
HC = H // P  # 8 h-chunks
SC = S // P  # 2 s-chunks
NOPT = 5

_CACHE: dict = {}
_label_hook = None


def _lbl(s):
    if _label_hook is not None:
        _label_hook(s)


def _build_bass(reps: int = 1, cfg: dict | None = None):
    cfg = dict(cfg or {})
    BF16_MODE = cfg.get("dtype", "bf16") == "bf16"
    NAT_BUFS = cfg.get("nat_bufs", 10)
    XT_BUFS = cfg.get("xt_bufs", NOPT + 2)
    WS_BUFS = cfg.get("ws_bufs", 5)
    E_BUFS = cfg.get("e_bufs", 5)
    OSB_BUFS = cfg.get("osb_bufs", 2)
    GP_SUB = cfg.get("gp_sub", False)
    PSM = cfg.get("ps_misc", 3)
    PSS = cfg.get("ps_s", 2)
    PSO = cfg.get("ps_o", 3)
    QBUFS = cfg.get("q_bufs", 4)
    from contextlib import ExitStack

    import concourse.mybir as mybir
    import concourse.tile as tile
    from concourse import bacc
    from concourse.masks import make_identity

    FP32 = mybir.dt.float32
    F32R = mybir.dt.float32r
    BF16 = mybir.dt.bfloat16
    DT = BF16 if BF16_MODE else F32R
    AF = mybir.ActivationFunctionType

    nc = bacc.Bacc(debug=bool(cfg.get('debug', False)))

    opt_d = [
        nc.dram_tensor(f"option{i + 1}", (BPC, S, H), DT, kind="ExternalInput")
        for i in range(NOPT)
    ]
    w_d = nc.dram_tensor("W", (H, H), DT, kind="ExternalInput")
    out_d = nc.dram_tensor("out", (BPC, S, H), FP32, kind="ExternalOutput")

    with ExitStack() as ctx:
        tc = ctx.enter_context(tile.TileContext(nc))
        const = ctx.enter_context(tc.tile_pool(name="const", bufs=1))
        natp = ctx.enter_context(tc.tile_pool(name="nat", bufs=NAT_BUFS))
        xp = ctx.enter_context(tc.tile_pool(name="xt", bufs=XT_BUFS))
        qp = ctx.enter_context(tc.tile_pool(name="qq", bufs=QBUFS))
        sp = ctx.enter_context(tc.tile_pool(name="ss", bufs=6))
        ep = ctx.enter_context(tc.tile_pool(name="ee", bufs=E_BUFS))
        mp_ = ctx.enter_context(tc.tile_pool(name="mm", bufs=2))
        zp = ctx.enter_context(tc.tile_pool(name="zz", bufs=2))
        rp = ctx.enter_context(tc.tile_pool(name="rr", bufs=2))
        wsp = ctx.enter_context(tc.tile_pool(name="wsum", bufs=WS_BUFS))
        tp = ctx.enter_context(tc.tile_pool(name="tmp", bufs=2))
        op_ = ctx.enter_context(tc.tile_pool(name="osb", bufs=OSB_BUFS))
        ps_misc = ctx.enter_context(tc.tile_pool(name="ps_misc", bufs=PSM, space="PSUM"))
        ps_s = ctx.enter_context(tc.tile_pool(name="ps_s", bufs=PSS, space="PSUM"))
        ps_o = ctx.enter_context(tc.tile_pool(name="ps_o", bufs=PSO, space="PSUM"))

        ident_f = const.tile([P, P], FP32)
        make_identity(nc, ident_f)
        ident = const.tile([P, P], DT)
        nc.vector.tensor_copy(out=ident, in_=ident_f)
        # W in 8 per-kc chunks so consumers only wait for the chunk they use
        w_sb = [const.tile([P, H], DT, name=f"w_{kc}") for kc in range(HC)]

        from contextlib import nullcontext

        loop_cm = tc.For_i(0, reps, 1) if reps > 1 else nullcontext()
        with loop_cm:
            # cross-batch carried prefetches
            carry = {"nat": {}, "x": {}, "q": {}}

            def load_nat(b, k, split=False):
                _lbl(f"load_nat b{b} k{k}")
                nk = natp.tile([P, SC, H], DT, tag="nat", name=f"nat_{b}_{k}")
                src_ap = opt_d[k].ap()[b].rearrange("(sc p) h -> p sc h", p=P)
                if split:
                    nc.sync.dma_start(out=nk[:, :, : H // 2],
                                      in_=src_ap[:, :, : H // 2])
                    nc.sync.dma_start(out=nk[:, :, H // 2 :],
                                      in_=src_ap[:, :, H // 2 :])
                else:
                    nc.sync.dma_start(out=nk, in_=src_ap)
                return nk

            def transpose_opt(b, k, nk):
                _lbl(f"transpose b{b} k{k}")
                xk = xp.tile([P, HC, S], DT, tag="xt", name=f"x_{b}_{k}")
                for j in range(HC // 2):  # pairs of h-chunks -> one PSUM bank
                    pt = ps_misc.tile([P, 4, P], DT, tag="ps_misc",
                                      name=f"pt_{b}_{k}_{j}")
                    for d in range(2):
                        hc = 2 * j + d
                        for sc in range(SC):
                            nc.tensor.transpose(
                                out=pt[:, 2 * d + sc, :],
                                in_=nk[:, sc, hc * P : (hc + 1) * P],
                                identity=ident,
                            )
                    dst = xk[:, 2 * j : 2 * j + 2, :]
                    if (k + j) % 2 == 0:
                        nc.scalar.copy(out=dst, in_=pt)
                    else:
                        nc.vector.tensor_copy(out=dst, in_=pt)
                return xk

            def emit_q(b, a, xa):
                _lbl(f"q b{b} a{a}")
                qt = qp.tile([P, HC, S], DT, tag="qq", name=f"q_{b}_{a}")
                pool, ptag = (ps_s, "ps_s") if a < 2 else (ps_misc, "ps_misc")
                for half in range(HC // 2):
                    pq = pool.tile([P, 2, S], FP32, tag=ptag,
                                   name=f"pq_{b}_{a}_{half}")
                    for d in range(2):
                        mc = 2 * half + d
                        for kc in range(HC):
                            nc.tensor.matmul(
                                pq[:, d, :],
                                w_sb[kc][:, mc * P : (mc + 1) * P],
                                xa[:, kc, :],
                                start=(kc == 0),
                                stop=(kc == HC - 1),
                            )
                    nc.scalar.copy(out=qt[:, 2 * half : 2 * half + 2, :], in_=pq)
                return qt

            for b in range(BPC):
                # ---- load options (steady state: all carried/prefetched) ----
                nat = []
                for k in range(NOPT):
                    nat.append(carry["nat"].get(k)
                               or load_nat(b, k, split=(b == 0 and k < 2)))
                if b == 0:
                    # W on the ACT hwdge ring so it never blocks option loads
                    for kc in range(HC):
                        nc.scalar.dma_start(
                            out=w_sb[kc],
                            in_=w_d.ap()[kc * P : (kc + 1) * P].rearrange(
                                "p h -> p h"),
                        )
                x = [carry["x"].get(k) for k in range(NOPT)]
                q = [carry["q"].get(a) for a in range(NOPT)]
                carry = {"nat": {}, "x": {}, "q": {}}

                def emit_scores(a):
                    s_sb = []
                    _lbl(f"scores b{b} a{a}")
                    for k in range(NOPT):
                        if k == a:
                            continue
                        st = ps_s.tile([P, SC, S], FP32, tag="ps_s",
                                       name=f"st_{b}_{a}_{k}")
                        for rc in range(SC):
                            for hc in range(HC):
                                nc.tensor.matmul(
                                    st[:, rc, :],
                                    x[k][:, hc, rc * P : (rc + 1) * P],
                                    q[a][:, hc, :],
                                    start=(hc == 0),
                                    stop=(hc == HC - 1),
                                )
                        ssb = sp.tile([P, SC, S], FP32, tag="ss",
                                      name=f"ssb_{b}_{a}_{k}")
                        if a == NOPT - 1 or len(s_sb) % 2 == 1:
                            nc.vector.tensor_copy(out=ssb, in_=st)
                        else:
                            nc.scalar.copy(out=ssb, in_=st)
                        s_sb.append(ssb)
                    return s_sb

                # wsum[k] accumulates sum_a softmax_weight(a, k): the output
                # matmul collapses to sum_k wsum_k @ opt_k (4x fewer matmuls)
                wsum = [None] * NOPT

                def emit_softmax(a, s_sb, split=False):
                    _lbl(f"softmax b{b} a{a}")
                    MUL = mybir.AluOpType.mult
                    m = mp_.tile([P, SC, S], FP32, tag="mm", name=f"m_{b}_{a}")
                    m2 = mp_.tile([P, SC, S], FP32, tag="m2", name=f"m2_{b}_{a}")
                    e = [ep.tile([P, SC, S], F32R, tag="ee",
                                 name=f"e_{b}_{a}_{k4}") for k4 in range(4)]
                    z = zp.tile([P, SC, S], FP32, tag="zz", name=f"z_{b}_{a}")
                    z23 = zp.tile([P, SC, S], FP32, tag="z2", name=f"z23_{b}_{a}")
                    rcp = rp.tile([P, SC, S], FP32, tag="rr", name=f"r_{b}_{a}")
                    ks = [k for k in range(NOPT) if k != a]
                    newk = [k4 for k4, k in enumerate(ks) if wsum[k] is None]
                    for k in ks:
                        if wsum[k] is None:
                            wsum[k] = wsp.tile([P, SC, S], DT, tag="wsum",
                                               name=f"ws_{b}_{k}")
                    rcs = range(SC) if split else [slice(None)]
                    for rc in rcs:
                        # max over the 4 options: tree split across DVE/Pool
                        nc.vector.tensor_max(m[:, rc], s_sb[0][:, rc],
                                             s_sb[1][:, rc])
                        nc.vector.tensor_max(m2[:, rc], s_sb[2][:, rc],
                                              s_sb[3][:, rc])
                        nc.vector.tensor_max(m[:, rc], m[:, rc], m2[:, rc])
                        for k4 in range(4):
                            sub_eng = nc.gpsimd if (GP_SUB and not split) \
                                else nc.vector
                            sub_eng.tensor_sub(s_sb[k4][:, rc],
                                               s_sb[k4][:, rc], m[:, rc])
                            nc.scalar.activation(out=e[k4][:, rc],
                                                 in_=s_sb[k4][:, rc],
                                                 func=AF.Exp)
                        nc.vector.tensor_add(z[:, rc], e[0][:, rc], e[1][:, rc])
                        nc.vector.tensor_add(z23[:, rc], e[2][:, rc],
                                               e[3][:, rc])
                        nc.vector.tensor_add(z[:, rc], z[:, rc], z23[:, rc])
                        nc.vector.reciprocal(rcp[:, rc], z[:, rc])
                        for k4, k in enumerate(ks):
                            # (e * 0.5) * rcp folds the final /2 for free
                            if k4 in newk:
                                nc.vector.scalar_tensor_tensor(
                                    wsum[k][:, rc], e[k4][:, rc], 0.5,
                                    rcp[:, rc], MUL, MUL)
                            else:
                                tmp = tp.tile([P, SC, S], DT, tag="tmp",
                                              name=f"t_{b}_{a}_{k4}")
                                nc.vector.scalar_tensor_tensor(
                                    tmp[:, rc], e[k4][:, rc], 0.5,
                                    rcp[:, rc], MUL, MUL)
                                nc.vector.tensor_add(wsum[k][:, rc],
                                                     wsum[k][:, rc],
                                                     tmp[:, rc])

                po = {}
                po_started = {}

                def emit_out_k(k, nn, last):
                    _lbl(f"AV b{b} k{k} nn{nn}")
                    for mp2 in range(SC):
                        key = (mp2, nn)
                        if key not in po:
                            po[key] = ps_o.tile([P, 512], FP32, tag="ps_o",
                                                name=f"po_{b}_{mp2}_{nn}")
                            po_started[key] = False
                        for rc in range(SC):
                            is_last = last and rc == SC - 1
                            nc.tensor.matmul(
                                po[key],
                                wsum[k][:, rc, mp2 * P : (mp2 + 1) * P],
                                nat[k][:, rc, nn * 512 : (nn + 1) * 512],
                                start=(not po_started[key]),
                                stop=is_last,
                            )
                            po_started[key] = True

                # ---- head: fill q pipeline (b==0: interleave with
                # transposes so the first q chases the W-chunk DMAs) ----
                for k in range(2):
                    if x[k] is None:
                        x[k] = transpose_opt(b, k, nat[k])
                if q[0] is None:
                    q[0] = emit_q(b, 0, x[0])
                for k in range(2, NOPT):
                    if x[k] is None:
                        x[k] = transpose_opt(b, k, nat[k])
                if q[1] is None:
                    q[1] = emit_q(b, 1, x[1])
                q[2] = emit_q(b, 2, x[2])
                s_cur = emit_scores(0)
                for a in range(NOPT):
                    if a + 3 < NOPT:
                        q[a + 3] = emit_q(b, a + 3, x[a + 3])
                    s_next = emit_scores(a + 1) if a + 1 < NOPT else None
                    if a == NOPT - 1 and b + 1 < BPC:
                        carry["x"][0] = transpose_opt(b + 1, 0, carry["nat"][0])
                        carry["x"][1] = transpose_opt(b + 1, 1, carry["nat"][1])
                        carry["q"][0] = emit_q(b + 1, 0, carry["x"][0])
                        carry["q"][1] = emit_q(b + 1, 1, carry["x"][1])
                        carry["x"][2] = transpose_opt(b + 1, 2, carry["nat"][2])
                        carry["x"][3] = transpose_opt(b + 1, 3, carry["nat"][3])
                        carry["x"][4] = transpose_opt(b + 1, 4, carry["nat"][4])
                    emit_softmax(a, s_cur,
                                 split=(b == BPC - 1 and a == NOPT - 1))
                    s_cur = s_next
                    # prefetch next item's options while scores stream
                    if b + 1 < BPC:
                        if a == 0:
                            carry["nat"][0] = load_nat(b + 1, 0)
                            carry["nat"][1] = load_nat(b + 1, 1)
                        elif a == 1:
                            carry["nat"][2] = load_nat(b + 1, 2)
                            carry["nat"][3] = load_nat(b + 1, 3)
                        elif a == 2:
                            carry["nat"][4] = load_nat(b + 1, 4)
                    if a == NOPT - 2:
                        # wsum for the last option is complete (it never
                        # scores against itself): overlap its out-matmuls
                        # with the final softmax
                        emit_out_k(NOPT - 1, 0, last=False)

                # ---- tail: AV phased by nn so only 2 out banks live
                # (except on the last item, where latency beats pressure) ----
                _lbl(f"osb b{b}")
                osb = op_.tile([P, SC, H], FP32, tag="osb", name=f"osb_{b}")
                last_item = b + 1 >= BPC
                if last_item:
                    phases = [(0, list(range(NOPT - 1))),
                              (1, list(range(NOPT)))]
                    for nn, ks in phases:
                        for k in ks:
                            emit_out_k(k, nn, last=(k == ks[-1]))
                    for nn, _ks in phases:
                        for mp2 in range(SC):
                            dst = osb[:, mp2, nn * 512 : (nn + 1) * 512]
                            if mp2 == 0:
                                nc.scalar.copy(out=dst, in_=po[(mp2, nn)])
                            else:
                                nc.vector.tensor_copy(out=dst, in_=po[(mp2, nn)])
                        nc.scalar.dma_start(
                            out=out_d.ap()[b].rearrange(
                                "(sc p) h -> p sc h", p=P)[:, :,
                                nn * 512 : (nn + 1) * 512],
                            in_=osb[:, :, nn * 512 : (nn + 1) * 512],
                        )
                else:
                    for nn in range(2):
                        ks = list(range(NOPT - 1)) + ([NOPT - 1] if nn == 1 else [])
                        for k in ks:
                            emit_out_k(k, nn, last=(k == ks[-1]))
                        for mp2 in range(SC):
                            dst = osb[:, mp2, nn * 512 : (nn + 1) * 512]
                            if mp2 == 0:
                                nc.scalar.copy(out=dst, in_=po[(mp2, nn)])
                            else:
                                nc.vector.tensor_copy(out=dst, in_=po[(mp2, nn)])
                        nc.scalar.dma_start(
                            out=out_d.ap()[b].rearrange(
                                "(sc p) h -> p sc h", p=P)[:, :,
                                nn * 512 : (nn + 1) * 512],
                            in_=osb[:, :, nn * 512 : (nn + 1) * 512],
                        )

    nc.compile()
    return nc


def _get_nc(reps: int = 1, cfg: dict | None = None):
    key = f"nc{reps}-{sorted((cfg or {}).items())}"
    if key not in _CACHE:
        _CACHE[key] = _build_bass(reps, cfg)
    return _CACHE[key]


def _in_dtype(cfg: dict | None = None):
    import ml_dtypes

    if (cfg or {}).get("dtype", "bf16") == "bf16":
        return ml_dtypes.bfloat16
    return np.float32


def kernel(**inputs) -> np.ndarray:
    from concourse.bass_utils import run_bass_kernel_spmd

    nc = _get_nc()
    dt = _in_dtype()
    opts = [np.ascontiguousarray(
        np.asarray(inputs[f"option{i + 1}"], dtype=np.float32).astype(dt))
        for i in range(NOPT)]
    W = np.ascontiguousarray(np.asarray(inputs["W"], dtype=np.float32).astype(dt))

    in_maps = []
    for c in range(NCORES):
        m = {f"option{i + 1}": opts[i][c * BPC : (c + 1) * BPC] for i in range(NOPT)}
        m["W"] = W
        in_maps.append(m)

    res = run_bass_kernel_spmd(nc, in_maps, list(range(NCORES)))
    out = np.concatenate([res.results[c]["out"] for c in range(NCORES)], axis=0)
    return np.asarray(out, dtype=np.float32)


# revision 22
# speedup vs baseline: 1.2066x; 1.0650x over previous
"""ChoiceAttention Trainium2 kernel.

Math (per batch item b, per "retain" iteration a over the 5 options):
    q_a = opt_a @ W                              (s, h)
    S_ak[p, r] = q_a[p, :] . opt_k[r, :]         for the 4 options k != a
    w_ak = softmax over k of (S_ak + bias)       (bias cancels: softmax is
                                                  shift-invariant over k)
    out += sum_k w_ak @ opt_k
final out /= 2.

Sharding: data-parallel over batch across 8 NeuronCores (4 items each),
W replicated. No collectives; host concatenates the per-core outputs.

Layout strategy per core / batch item:
    nat_k : opt_k natural layout      (128p, 2 sc, 1024h)  - DMA'd in (bf16)
    x_k   : opt_k transposed (h-major)(128p, 8 hc, 256s)   - PE transposes
    q_a^T : h-major q                 (128p, 8 hc, 256s)   - matmul(W, x_a)
    S_ak^T: scores transposed         (128p, 2 rc, 256p)   - matmul(x_k, q_a^T)
    softmax over the four k tiles elementwise (max-subtract, exp, recip)
    out   : accumulated in 4 PSUM banks over all 40 (a,k,rc) matmul groups
Matmul operands are bf16; scores/softmax accumulate in fp32.

Software pipeline (steady state): item b's tail overlaps the final softmax
and AV matmuls with item b+1's first transposes and q-projections; W is
loaded in 8 per-kc chunks so the first q can chase the DMA.
"""

import numpy as np

B, S, H = 32, 256, 1024
NCORES = 8
BPC = B // NCORES  # batch items per core
P = 128
HC = H // P  # 8 h-chunks
SC = S // P  # 2 s-chunks
NOPT = 5

_CACHE: dict = {}
_label_hook = None


def _lbl(s):
    if _label_hook is not None:
        _label_hook(s)


def _build_bass(reps: int = 1, cfg: dict | None = None):
    cfg = dict(cfg or {})
    BF16_MODE = cfg.get("dtype", "bf16") == "bf16"
    NAT_BUFS = cfg.get("nat_bufs", 10)
    XT_BUFS = cfg.get("xt_bufs", NOPT + 2)
    WS_BUFS = cfg.get("ws_bufs", 5)
    E_BUFS = cfg.get("e_bufs", 5)
    OSB_BUFS = cfg.get("osb_bufs", 2)
    GP_SUB = cfg.get("gp_sub", False)
    PSM = cfg.get("ps_misc", 3)
    PSS = cfg.get("ps_s", 2)
    PSO = cfg.get("ps_o", 3)
    QBUFS = cfg.get("q_bufs", 4)
    SPBUFS = cfg.get("sp_bufs", 6)
    TAIL_SCALAR = cfg.get("tail_scalar", False)
    from contextlib import ExitStack

    import concourse.mybir as mybir
    import concourse.tile as tile
    from concourse import bacc
    from concourse.masks import make_identity

    FP32 = mybir.dt.float32
    F32R = mybir.dt.float32r
    BF16 = mybir.dt.bfloat16
    FP16 = mybir.dt.float16
    DT = BF16 if BF16_MODE else F32R
    AF = mybir.ActivationFunctionType

    nc = bacc.Bacc(debug=bool(cfg.get('debug', False)))

    opt_d = [
        nc.dram_tensor(f"option{i + 1}", (BPC, S, H), DT, kind="ExternalInput")
        for i in range(NOPT)
    ]
    w_d = nc.dram_tensor("W", (H, H), DT, kind="ExternalInput")
    out_d = nc.dram_tensor("out", (BPC, S, H), FP32, kind="ExternalOutput")

    with ExitStack() as ctx:
        tc = ctx.enter_context(tile.TileContext(nc))
        const = ctx.enter_context(tc.tile_pool(name="const", bufs=1))
        natp = ctx.enter_context(tc.tile_pool(name="nat", bufs=NAT_BUFS))
        xp = ctx.enter_context(tc.tile_pool(name="xt", bufs=XT_BUFS))
        qp = ctx.enter_context(tc.tile_pool(name="qq", bufs=QBUFS))
        sp = ctx.enter_context(tc.tile_pool(name="ss", bufs=SPBUFS))
        ep = ctx.enter_context(tc.tile_pool(name="ee", bufs=E_BUFS))
        mp_ = ctx.enter_context(tc.tile_pool(name="mm", bufs=2))
        zp = ctx.enter_context(tc.tile_pool(name="zz", bufs=2))
        rp = ctx.enter_context(tc.tile_pool(name="rr", bufs=2))
        wsp = ctx.enter_context(tc.tile_pool(name="wsum", bufs=WS_BUFS))
        tp = ctx.enter_context(tc.tile_pool(name="tmp", bufs=2))
        op_ = ctx.enter_context(tc.tile_pool(name="osb", bufs=OSB_BUFS))
        ps_misc = ctx.enter_context(tc.tile_pool(name="ps_misc", bufs=PSM, space="PSUM"))
        ps_s = ctx.enter_context(tc.tile_pool(name="ps_s", bufs=PSS, space="PSUM"))
        ps_o = ctx.enter_context(tc.tile_pool(name="ps_o", bufs=PSO, space="PSUM"))

        ident_f = const.tile([P, P], FP32)
        make_identity(nc, ident_f)
        ident = const.tile([P, P], DT)
        nc.vector.tensor_copy(out=ident, in_=ident_f)
        # W in 8 per-kc chunks so consumers only wait for the chunk they use
        w_sb = [const.tile([P, H], DT, name=f"w_{kc}") for kc in range(HC)]

        from contextlib import nullcontext

        loop_cm = tc.For_i(0, reps, 1) if reps > 1 else nullcontext()
        with loop_cm:
            # cross-batch carried prefetches
            carry = {"nat": {}, "x": {}, "q": {}}

            def load_nat(b, k, split=False):
                _lbl(f"load_nat b{b} k{k}")
                nk = natp.tile([P, SC, H], DT, tag="nat", name=f"nat_{b}_{k}")
                src_ap = opt_d[k].ap()[b].rearrange("(sc p) h -> p sc h", p=P)
                if split:
                    nc.sync.dma_start(out=nk[:, :, : H // 2],
                                      in_=src_ap[:, :, : H // 2])
                    nc.sync.dma_start(out=nk[:, :, H // 2 :],
                                      in_=src_ap[:, :, H // 2 :])
                else:
                    nc.sync.dma_start(out=nk, in_=src_ap)
                return nk

            def transpose_opt(b, k, nk, tail=False):
                _lbl(f"transpose b{b} k{k}")
                xk = xp.tile([P, HC, S], DT, tag="xt", name=f"x_{b}_{k}")
                for j in range(HC // 2):  # pairs of h-chunks -> one PSUM bank
                    pt = ps_misc.tile([P, 4, P], DT, tag="ps_misc",
                                      name=f"pt_{b}_{k}_{j}")
                    for d in range(2):
                        hc = 2 * j + d
                        for sc in range(SC):
                            nc.tensor.transpose(
                                out=pt[:, 2 * d + sc, :],
                                in_=nk[:, sc, hc * P : (hc + 1) * P],
                                identity=ident,
                            )
                    dst = xk[:, 2 * j : 2 * j + 2, :]
                    if (tail and TAIL_SCALAR) or (
                            not (tail and TAIL_SCALAR) and (k + j) % 2 == 0):
                        nc.scalar.copy(out=dst, in_=pt)
                    else:
                        nc.vector.tensor_copy(out=dst, in_=pt)
                return xk

            def emit_q(b, a, xa):
                _lbl(f"q b{b} a{a}")
                qt = qp.tile([P, HC, S], DT, tag="qq", name=f"q_{b}_{a}")
                use_ss = (a < 2) if cfg.get("q_pool", "split") == "split" else False
                pool, ptag = (ps_s, "ps_s") if use_ss else (ps_misc, "ps_misc")
                for half in range(HC // 2):
                    pq = pool.tile([P, 2, S], FP32, tag=ptag,
                                   name=f"pq_{b}_{a}_{half}")
                    for d in range(2):
                        mc = 2 * half + d
                        for kc in range(HC):
                            nc.tensor.matmul(
                                pq[:, d, :],
                                w_sb[kc][:, mc * P : (mc + 1) * P],
                                xa[:, kc, :],
                                start=(kc == 0),
                                stop=(kc == HC - 1),
                            )
                    nc.scalar.copy(out=qt[:, 2 * half : 2 * half + 2, :], in_=pq)
                return qt

            for b in range(BPC):
                # ---- load options (steady state: all carried/prefetched) ----
                nat = []
                for k in range(NOPT):
                    nat.append(carry["nat"].get(k) or load_nat(b, k))
                if b == 0:
                    # W on the ACT hwdge ring so it never blocks option loads
                    for kc in range(HC):
                        nc.scalar.dma_start(
                            out=w_sb[kc],
                            in_=w_d.ap()[kc * P : (kc + 1) * P].rearrange(
                                "p h -> p h"),
                        )
                x = [carry["x"].get(k) for k in range(NOPT)]
                q = [carry["q"].get(a) for a in range(NOPT)]
                carry = {"nat": {}, "x": {}, "q": {}}

                def emit_scores(a):
                    s_sb = []
                    _lbl(f"scores b{b} a{a}")
                    for k in range(NOPT):
                        if k == a:
                            continue
                        st = ps_s.tile([P, SC, S], FP32, tag="ps_s",
                                       name=f"st_{b}_{a}_{k}")
                        for rc in range(SC):
                            for hc in range(HC):
                                nc.tensor.matmul(
                                    st[:, rc, :],
                                    x[k][:, hc, rc * P : (rc + 1) * P],
                                    q[a][:, hc, :],
                                    start=(hc == 0),
                                    stop=(hc == HC - 1),
                                )
                        ssb = sp.tile([P, SC, S], FP16, tag="ss",
                                      name=f"ssb_{b}_{a}_{k}")
                        if len(s_sb) % 2 == 1:
                            nc.vector.tensor_copy(out=ssb, in_=st)
                        else:
                            nc.scalar.copy(out=ssb, in_=st)
                        s_sb.append(ssb)
                    return s_sb

                # wsum[k] accumulates sum_a softmax_weight(a, k): the output
                # matmul collapses to sum_k wsum_k @ opt_k (4x fewer matmuls)
                wsum = [None] * NOPT

                def emit_softmax(a, s_sb, split=False):
                    _lbl(f"softmax b{b} a{a}")
                    MUL = mybir.AluOpType.mult
                    m = mp_.tile([P, SC, S], FP16, tag="mm", name=f"m_{b}_{a}")
                    m2 = mp_.tile([P, SC, S], FP16, tag="m2", name=f"m2_{b}_{a}")
                    e = [ep.tile([P, SC, S], FP16, tag="ee",
                                 name=f"e_{b}_{a}_{k4}") for k4 in range(4)]
                    z = zp.tile([P, SC, S], FP16, tag="zz", name=f"z_{b}_{a}")
                    z23 = zp.tile([P, SC, S], FP16, tag="z2", name=f"z23_{b}_{a}")
                    rcp = rp.tile([P, SC, S], FP16, tag="rr", name=f"r_{b}_{a}")
                    ks = [k for k in range(NOPT) if k != a]
                    newk = [k4 for k4, k in enumerate(ks) if wsum[k] is None]
                    for k in ks:
                        if wsum[k] is None:
                            wsum[k] = wsp.tile([P, SC, S], DT, tag="wsum",
                                               name=f"ws_{b}_{k}")
                    rcs = range(SC) if split else [slice(None)]
                    for rc in rcs:
                        # max over the 4 options: tree split across DVE/Pool
                        nc.vector.tensor_max(m[:, rc], s_sb[0][:, rc],
                                             s_sb[1][:, rc])
                        nc.vector.tensor_max(m2[:, rc], s_sb[2][:, rc],
                                              s_sb[3][:, rc])
                        nc.vector.tensor_max(m[:, rc], m[:, rc], m2[:, rc])
                        for k4 in range(4):
                            sub_eng = nc.gpsimd if (GP_SUB and not split) \
                                else nc.vector
                            sub_eng.tensor_sub(s_sb[k4][:, rc],
                                               s_sb[k4][:, rc], m[:, rc])
                            nc.scalar.activation(out=e[k4][:, rc],
                                                 in_=s_sb[k4][:, rc],
                                                 func=AF.Exp)
                        nc.vector.tensor_add(z[:, rc], e[0][:, rc], e[1][:, rc])
                        nc.vector.tensor_add(z23[:, rc], e[2][:, rc],
                                               e[3][:, rc])
                        nc.vector.tensor_add(z[:, rc], z[:, rc], z23[:, rc])
                        with nc.allow_low_precision(
                                reason="z in [1,4], fp16 recip err ~1e-3"):
                            nc.vector.reciprocal(rcp[:, rc], z[:, rc])
                        for k4, k in enumerate(ks):
                            # (e * 0.5) * rcp folds the final /2 for free
                            if k4 in newk:
                                nc.vector.scalar_tensor_tensor(
                                    wsum[k][:, rc], e[k4][:, rc], 0.5,
                                    rcp[:, rc], MUL, MUL)
                            else:
                                tmp = tp.tile([P, SC, S], DT, tag="tmp",
                                              name=f"t_{b}_{a}_{k4}")
                                nc.vector.scalar_tensor_tensor(
                                    tmp[:, rc], e[k4][:, rc], 0.5,
                                    rcp[:, rc], MUL, MUL)
                                nc.vector.tensor_add(wsum[k][:, rc],
                                                     wsum[k][:, rc],
                                                     tmp[:, rc])

                po = {}
                po_started = {}

                def emit_out_k(k, nn, last):
                    _lbl(f"AV b{b} k{k} nn{nn}")
                    for mp2 in range(SC):
                        key = (mp2, nn)
                        if key not in po:
                            po[key] = ps_o.tile([P, 512], FP32, tag="ps_o",
                                                name=f"po_{b}_{mp2}_{nn}")
                            po_started[key] = False
                        for rc in range(SC):
                            is_last = last and rc == SC - 1
                            nc.tensor.matmul(
                                po[key],
                                wsum[k][:, rc, mp2 * P : (mp2 + 1) * P],
                                nat[k][:, rc, nn * 512 : (nn + 1) * 512],
                                start=(not po_started[key]),
                                stop=is_last,
                            )
                            po_started[key] = True

                # ---- head: fill q pipeline (b==0: interleave with
                # transposes so the first q chases the W-chunk DMAs) ----
                for k in range(2):
                    if x[k] is None:
                        x[k] = transpose_opt(b, k, nat[k])
                if q[0] is None:
                    q[0] = emit_q(b, 0, x[0])
                for k in range(2, NOPT):
                    if x[k] is None:
                        x[k] = transpose_opt(b, k, nat[k])
                if q[1] is None:
                    q[1] = emit_q(b, 1, x[1])
                q[2] = emit_q(b, 2, x[2])
                s_cur = emit_scores(0)
                for a in range(NOPT):
                    if a + 3 < NOPT:
                        q[a + 3] = emit_q(b, a + 3, x[a + 3])
                    s_next = emit_scores(a + 1) if a + 1 < NOPT else None
                    if a == NOPT - 2 and b + 1 < BPC:
                        # emit next item's transposes + first q-projections
                        # here: their ~22us of PE work hides the latency of
                        # BOTH remaining softmax chains
                        carry["x"][0] = transpose_opt(
                            b + 1, 0, carry["nat"][0], tail=True)
                        carry["x"][1] = transpose_opt(
                            b + 1, 1, carry["nat"][1], tail=True)
                        carry["q"][0] = emit_q(b + 1, 0, carry["x"][0])
                        carry["q"][1] = emit_q(b + 1, 1, carry["x"][1])
                        carry["x"][2] = transpose_opt(b + 1, 2, carry["nat"][2])
                        carry["x"][3] = transpose_opt(b + 1, 3, carry["nat"][3])
                        carry["x"][4] = transpose_opt(b + 1, 4, carry["nat"][4])
                    emit_softmax(a, s_cur,
                                 split=(b == BPC - 1 and a == NOPT - 1))
                    s_cur = s_next
                    # prefetch next item's options while scores stream
                    if b + 1 < BPC:
                        if a == 0:
                            carry["nat"][0] = load_nat(b + 1, 0)
                            carry["nat"][1] = load_nat(b + 1, 1)
                        elif a == 1:
                            carry["nat"][2] = load_nat(b + 1, 2)
                            carry["nat"][3] = load_nat(b + 1, 3)
                        elif a == 2:
                            carry["nat"][4] = load_nat(b + 1, 4)
                    if a == NOPT - 2:
                        # wsum for the last option is complete (it never
                        # scores against itself): overlap its out-matmuls
                        # with the final softmax
                        emit_out_k(NOPT - 1, 0, last=False)

                # ---- tail: AV phased by nn so only 2 out banks live
                # (except on the last item, where latency beats pressure) ----
                _lbl(f"osb b{b}")
                osb = op_.tile([P, SC, H], FP32, tag="osb", name=f"osb_{b}")
                last_item = False
                if last_item:
                    phases = [(0, list(range(NOPT - 1))),
                              (1, list(range(NOPT)))]
                    for nn, ks in phases:
                        for k in ks:
                            emit_out_k(k, nn, last=(k == ks[-1]))
                    for nn, _ks in phases:
                        for mp2 in range(SC):
                            dst = osb[:, mp2, nn * 512 : (nn + 1) * 512]
                            if mp2 == 0:
                                nc.scalar.copy(out=dst, in_=po[(mp2, nn)])
                            else:
                                nc.vector.tensor_copy(out=dst, in_=po[(mp2, nn)])
                        nc.scalar.dma_start(
                            out=out_d.ap()[b].rearrange(
                                "(sc p) h -> p sc h", p=P)[:, :,
                                nn * 512 : (nn + 1) * 512],
                            in_=osb[:, :, nn * 512 : (nn + 1) * 512],
                        )
                else:
                    for nn in range(2):
                        ks = list(range(NOPT - 1)) + ([NOPT - 1] if nn == 1 else [])
                        for k in ks:
                            emit_out_k(k, nn, last=(k == ks[-1]))
                        for mp2 in range(SC):
                            dst = osb[:, mp2, nn * 512 : (nn + 1) * 512]
                            if mp2 == 0:
                                nc.scalar.copy(out=dst, in_=po[(mp2, nn)])
                            else:
                                nc.vector.tensor_copy(out=dst, in_=po[(mp2, nn)])
                        nc.scalar.dma_start(
                            out=out_d.ap()[b].rearrange(
                                "(sc p) h -> p sc h", p=P)[:, :,
                                nn * 512 : (nn + 1) * 512],
                            in_=osb[:, :, nn * 512 : (nn + 1) * 512],
                        )

    nc.compile()
    return nc


def _get_nc(reps: int = 1, cfg: dict | None = None):
    key = f"nc{reps}-{sorted((cfg or {}).items())}"
    if key not in _CACHE:
        _CACHE[key] = _build_bass(reps, cfg)
    return _CACHE[key]


def _in_dtype(cfg: dict | None = None):
    import ml_dtypes

    if (cfg or {}).get("dtype", "bf16") == "bf16":
        return ml_dtypes.bfloat16
    return np.float32


def kernel(**inputs) -> np.ndarray:
    from concourse.bass_utils import run_bass_kernel_spmd

    nc = _get_nc()
    dt = _in_dtype()
    opts = [np.ascontiguousarray(
        np.asarray(inputs[f"option{i + 1}"], dtype=np.float32).astype(dt))
        for i in range(NOPT)]
    W = np.ascontiguousarray(np.asarray(inputs["W"], dtype=np.float32).astype(dt))

    in_maps = []
    for c in range(NCORES):
        m = {f"option{i + 1}": opts[i][c * BPC : (c + 1) * BPC] for i in range(NOPT)}
        m["W"] = W
        in_maps.append(m)

    res = run_bass_kernel_spmd(nc, in_maps, list(range(NCORES)))
    out = np.concatenate([res.results[c]["out"] for c in range(NCORES)], axis=0)
    return np.asarray(out, dtype=np.float32)


# revision 26
# speedup vs baseline: 1.2080x; 1.0012x over previous
"""ChoiceAttention Trainium2 kernel.

Math (per batch item b, per "retain" iteration a over the 5 options):
    q_a = opt_a @ W                              (s, h)
    S_ak[p, r] = q_a[p, :] . opt_k[r, :]         for the 4 options k != a
    w_ak = softmax over k of (S_ak + bias)       (bias cancels: softmax is
                                                  shift-invariant over k)
    out += sum_k w_ak @ opt_k
final out /= 2.

Sharding: data-parallel over batch across 8 NeuronCores (4 items each),
W replicated. No collectives; host concatenates the per-core outputs.

Layout strategy per core / batch item:
    nat_k : opt_k natural layout      (128p, 2 sc, 1024h)  - DMA'd in (bf16)
    x_k   : opt_k transposed (h-major)(128p, 8 hc, 256s)   - PE transposes
    q_a^T : h-major q                 (128p, 8 hc, 256s)   - matmul(W, x_a)
    S_ak^T: scores transposed         (128p, 2 rc, 256p)   - matmul(x_k, q_a^T)
    softmax over the four k tiles elementwise (max-subtract, exp, recip)
    out   : accumulated in 4 PSUM banks over all 40 (a,k,rc) matmul groups
Matmul operands are bf16; scores/softmax accumulate in fp32.

Software pipeline (steady state): item b's tail overlaps the final softmax
and AV matmuls with item b+1's first transposes and q-projections; W is
loaded in 8 per-kc chunks so the first q can chase the DMA.
"""

import numpy as np

B, S, H = 32, 256, 1024
NCORES = 8
BPC = B // NCORES  # batch items per core
P = 128
HC = H // P  # 8 h-chunks
SC = S // P  # 2 s-chunks
NOPT = 5

_CACHE: dict = {}
_label_hook = None


def _lbl(s):
    if _label_hook is not None:
        _label_hook(s)


def _build_bass(reps: int = 1, cfg: dict | None = None):
    cfg = dict(cfg or {})
    BF16_MODE = cfg.get("dtype", "bf16") == "bf16"
    NAT_BUFS = cfg.get("nat_bufs", 10)
    XT_BUFS = cfg.get("xt_bufs", NOPT + 2)
    WS_BUFS = cfg.get("ws_bufs", 5)
    E_BUFS = cfg.get("e_bufs", 5)
    OSB_BUFS = cfg.get("osb_bufs", 2)
    GP_SUB = cfg.get("gp_sub", False)
    PSM = cfg.get("ps_misc", 4)
    PSS = cfg.get("ps_s", 2)
    PSO = cfg.get("ps_o", 2)
    QBUFS = cfg.get("q_bufs", 4)
    SPBUFS = cfg.get("sp_bufs", 8)
    TAIL_SCALAR = cfg.get("tail_scalar", False)
    from contextlib import ExitStack

    import concourse.mybir as mybir
    import concourse.tile as tile
    from concourse import bacc
    from concourse.masks import make_identity

    FP32 = mybir.dt.float32
    F32R = mybir.dt.float32r
    BF16 = mybir.dt.bfloat16
    FP16 = mybir.dt.float16
    DT = BF16 if BF16_MODE else F32R
    AF = mybir.ActivationFunctionType

    nc = bacc.Bacc(debug=bool(cfg.get('debug', False)))

    opt_d = [
        nc.dram_tensor(f"option{i + 1}", (BPC, S, H), DT, kind="ExternalInput")
        for i in range(NOPT)
    ]
    w_d = nc.dram_tensor("W", (H, H), DT, kind="ExternalInput")
    out_d = nc.dram_tensor("out", (BPC, S, H), FP32, kind="ExternalOutput")

    with ExitStack() as ctx:
        tc = ctx.enter_context(tile.TileContext(nc))
        const = ctx.enter_context(tc.tile_pool(name="const", bufs=1))
        natp = ctx.enter_context(tc.tile_pool(name="nat", bufs=NAT_BUFS))
        xp = ctx.enter_context(tc.tile_pool(name="xt", bufs=XT_BUFS))
        qp = ctx.enter_context(tc.tile_pool(name="qq", bufs=QBUFS))
        sp = ctx.enter_context(tc.tile_pool(name="ss", bufs=SPBUFS))
        ep = ctx.enter_context(tc.tile_pool(name="ee", bufs=E_BUFS))
        mp_ = ctx.enter_context(tc.tile_pool(name="mm", bufs=2))
        zp = ctx.enter_context(tc.tile_pool(name="zz", bufs=2))
        rp = ctx.enter_context(tc.tile_pool(name="rr", bufs=2))
        wsp = ctx.enter_context(tc.tile_pool(name="wsum", bufs=WS_BUFS))
        tp = ctx.enter_context(tc.tile_pool(name="tmp", bufs=2))
        op_ = ctx.enter_context(tc.tile_pool(name="osb", bufs=OSB_BUFS))
        ps_misc = ctx.enter_context(tc.tile_pool(name="ps_misc", bufs=PSM, space="PSUM"))
        ps_s = ctx.enter_context(tc.tile_pool(name="ps_s", bufs=PSS, space="PSUM"))
        ps_o = ctx.enter_context(tc.tile_pool(name="ps_o", bufs=PSO, space="PSUM"))

        ident_f = const.tile([P, P], FP32)
        make_identity(nc, ident_f)
        ident = const.tile([P, P], DT)
        nc.vector.tensor_copy(out=ident, in_=ident_f)
        # W in 8 per-kc chunks so consumers only wait for the chunk they use
        w_sb = [const.tile([P, H], DT, name=f"w_{kc}") for kc in range(HC)]

        from contextlib import nullcontext

        loop_cm = tc.For_i(0, reps, 1) if reps > 1 else nullcontext()
        with loop_cm:
            # cross-batch carried prefetches
            carry = {"nat": {}, "x": {}, "q": {}}

            def load_nat(b, k, split=False):
                _lbl(f"load_nat b{b} k{k}")
                nk = natp.tile([P, SC, H], DT, tag="nat", name=f"nat_{b}_{k}")
                src_ap = opt_d[k].ap()[b].rearrange("(sc p) h -> p sc h", p=P)
                if split:
                    nc.sync.dma_start(out=nk[:, :, : H // 2],
                                      in_=src_ap[:, :, : H // 2])
                    nc.sync.dma_start(out=nk[:, :, H // 2 :],
                                      in_=src_ap[:, :, H // 2 :])
                else:
                    nc.sync.dma_start(out=nk, in_=src_ap)
                return nk

            def transpose_opt(b, k, nk, tail=False):
                _lbl(f"transpose b{b} k{k}")
                xk = xp.tile([P, HC, S], DT, tag="xt", name=f"x_{b}_{k}")
                for j in range(HC // 2):  # pairs of h-chunks -> one PSUM bank
                    pt = ps_misc.tile([P, 4, P], DT, tag="ps_misc",
                                      name=f"pt_{b}_{k}_{j}")
                    for d in range(2):
                        hc = 2 * j + d
                        for sc in range(SC):
                            nc.tensor.transpose(
                                out=pt[:, 2 * d + sc, :],
                                in_=nk[:, sc, hc * P : (hc + 1) * P],
                                identity=ident,
                            )
                    dst = xk[:, 2 * j : 2 * j + 2, :]
                    if (tail and TAIL_SCALAR) or (
                            not (tail and TAIL_SCALAR) and (k + j) % 2 == 0):
                        nc.scalar.copy(out=dst, in_=pt)
                    else:
                        nc.vector.tensor_copy(out=dst, in_=pt)
                return xk

            def emit_q(b, a, xa):
                _lbl(f"q b{b} a{a}")
                qt = qp.tile([P, HC, S], DT, tag="qq", name=f"q_{b}_{a}")
                use_ss = (a < 2) if cfg.get("q_pool", "split") == "split" else False
                pool, ptag = (ps_s, "ps_s") if use_ss else (ps_misc, "ps_misc")
                for half in range(HC // 2):
                    pq = pool.tile([P, 2, S], FP32, tag=ptag,
                                   name=f"pq_{b}_{a}_{half}")
                    for d in range(2):
                        mc = 2 * half + d
                        for kc in range(HC):
                            nc.tensor.matmul(
                                pq[:, d, :],
                                w_sb[kc][:, mc * P : (mc + 1) * P],
                                xa[:, kc, :],
                                start=(kc == 0),
                                stop=(kc == HC - 1),
                            )
                    nc.scalar.copy(out=qt[:, 2 * half : 2 * half + 2, :], in_=pq)
                return qt

            for b in range(BPC):
                # ---- load options (steady state: all carried/prefetched) ----
                nat = []
                for k in range(NOPT):
                    nat.append(carry["nat"].get(k) or load_nat(b, k))
                if b == 0:
                    # W on the ACT hwdge ring so it never blocks option loads
                    for kc in range(HC):
                        nc.scalar.dma_start(
                            out=w_sb[kc],
                            in_=w_d.ap()[kc * P : (kc + 1) * P].rearrange(
                                "p h -> p h"),
                        )
                x = [carry["x"].get(k) for k in range(NOPT)]
                q = [carry["q"].get(a) for a in range(NOPT)]
                carry = {"nat": {}, "x": {}, "q": {}}

                def emit_scores(a):
                    s_sb = []
                    _lbl(f"scores b{b} a{a}")
                    for k in range(NOPT):
                        if k == a:
                            continue
                        st = ps_s.tile([P, SC, S], FP32, tag="ps_s",
                                       name=f"st_{b}_{a}_{k}")
                        for rc in range(SC):
                            for hc in range(HC):
                                nc.tensor.matmul(
                                    st[:, rc, :],
                                    x[k][:, hc, rc * P : (rc + 1) * P],
                                    q[a][:, hc, :],
                                    start=(hc == 0),
                                    stop=(hc == HC - 1),
                                )
                        ssb = sp.tile([P, SC, S], FP16, tag="ss",
                                      name=f"ssb_{b}_{a}_{k}")
                        if len(s_sb) % 2 == 1:
                            nc.vector.tensor_copy(out=ssb, in_=st)
                        else:
                            nc.scalar.copy(out=ssb, in_=st)
                        s_sb.append(ssb)
                    return s_sb

                # wsum[k] accumulates sum_a softmax_weight(a, k): the output
                # matmul collapses to sum_k wsum_k @ opt_k (4x fewer matmuls)
                wsum = [None] * NOPT

                def emit_softmax(a, s_sb, split=False):
                    _lbl(f"softmax b{b} a{a}")
                    MUL = mybir.AluOpType.mult
                    m = mp_.tile([P, SC, S], FP16, tag="mm", name=f"m_{b}_{a}")
                    m2 = mp_.tile([P, SC, S], FP16, tag="m2", name=f"m2_{b}_{a}")
                    e = [ep.tile([P, SC, S], FP16, tag="ee",
                                 name=f"e_{b}_{a}_{k4}") for k4 in range(4)]
                    z = zp.tile([P, SC, S], FP16, tag="zz", name=f"z_{b}_{a}")
                    z23 = zp.tile([P, SC, S], FP16, tag="z2", name=f"z23_{b}_{a}")
                    rcp = rp.tile([P, SC, S], FP16, tag="rr", name=f"r_{b}_{a}")
                    ks = [k for k in range(NOPT) if k != a]
                    newk = [k4 for k4, k in enumerate(ks) if wsum[k] is None]
                    for k in ks:
                        if wsum[k] is None:
                            wsum[k] = wsp.tile([P, SC, S], DT, tag="wsum",
                                               name=f"ws_{b}_{k}")
                    rcs = range(SC) if split else [slice(None)]
                    for rc in rcs:
                        # max over the 4 options: tree split across DVE/Pool
                        nc.vector.tensor_max(m[:, rc], s_sb[0][:, rc],
                                             s_sb[1][:, rc])
                        nc.vector.tensor_max(m2[:, rc], s_sb[2][:, rc],
                                              s_sb[3][:, rc])
                        nc.vector.tensor_max(m[:, rc], m[:, rc], m2[:, rc])
                        for k4 in range(4):
                            sub_eng = nc.gpsimd if (GP_SUB and not split) \
                                else nc.vector
                            sub_eng.tensor_sub(s_sb[k4][:, rc],
                                               s_sb[k4][:, rc], m[:, rc])
                            nc.scalar.activation(out=e[k4][:, rc],
                                                 in_=s_sb[k4][:, rc],
                                                 func=AF.Exp)
                        nc.vector.tensor_add(z[:, rc], e[0][:, rc], e[1][:, rc])
                        nc.vector.tensor_add(z23[:, rc], e[2][:, rc],
                                               e[3][:, rc])
                        nc.vector.tensor_add(z[:, rc], z[:, rc], z23[:, rc])
                        with nc.allow_low_precision(
                                reason="z in [1,4], fp16 recip err ~1e-3"):
                            nc.vector.reciprocal(rcp[:, rc], z[:, rc])
                        for k4, k in enumerate(ks):
                            # (e * 0.5) * rcp folds the final /2 for free
                            if k4 in newk:
                                nc.vector.scalar_tensor_tensor(
                                    wsum[k][:, rc], e[k4][:, rc], 0.5,
                                    rcp[:, rc], MUL, MUL)
                            else:
                                tmp = tp.tile([P, SC, S], DT, tag="tmp",
                                              name=f"t_{b}_{a}_{k4}")
                                nc.vector.scalar_tensor_tensor(
                                    tmp[:, rc], e[k4][:, rc], 0.5,
                                    rcp[:, rc], MUL, MUL)
                                nc.vector.tensor_add(wsum[k][:, rc],
                                                     wsum[k][:, rc],
                                                     tmp[:, rc])

                po = {}
                po_started = {}

                def emit_out_k(k, nn, last):
                    _lbl(f"AV b{b} k{k} nn{nn}")
                    for mp2 in range(SC):
                        key = (mp2, nn)
                        if key not in po:
                            po[key] = ps_o.tile([P, 512], FP32, tag="ps_o",
                                                name=f"po_{b}_{mp2}_{nn}")
                            po_started[key] = False
                        for rc in range(SC):
                            is_last = last and rc == SC - 1
                            nc.tensor.matmul(
                                po[key],
                                wsum[k][:, rc, mp2 * P : (mp2 + 1) * P],
                                nat[k][:, rc, nn * 512 : (nn + 1) * 512],
                                start=(not po_started[key]),
                                stop=is_last,
                            )
                            po_started[key] = True

                # ---- head: fill q pipeline (b==0: interleave with
                # transposes so the first q chases the W-chunk DMAs) ----
                for k in range(2):
                    if x[k] is None:
                        x[k] = transpose_opt(b, k, nat[k])
                if q[0] is None:
                    q[0] = emit_q(b, 0, x[0])
                for k in range(2, NOPT):
                    if x[k] is None:
                        x[k] = transpose_opt(b, k, nat[k])
                if q[1] is None:
                    q[1] = emit_q(b, 1, x[1])
                if q[2] is None:
                    q[2] = emit_q(b, 2, x[2])
                s_cur = emit_scores(0)
                for a in range(NOPT):
                    if a + 3 < NOPT:
                        q[a + 3] = emit_q(b, a + 3, x[a + 3])
                    s_next = emit_scores(a + 1) if a + 1 < NOPT else None
                    if a == NOPT - 2 and b + 1 < BPC:
                        # emit next item's transposes + first q-projections
                        # here: their ~22us of PE work hides the latency of
                        # BOTH remaining softmax chains
                        carry["x"][0] = transpose_opt(
                            b + 1, 0, carry["nat"][0], tail=True)
                        carry["x"][1] = transpose_opt(
                            b + 1, 1, carry["nat"][1], tail=True)
                        carry["q"][0] = emit_q(b + 1, 0, carry["x"][0])
                        carry["q"][1] = emit_q(b + 1, 1, carry["x"][1])
                        carry["x"][2] = transpose_opt(b + 1, 2, carry["nat"][2])
                        carry["x"][3] = transpose_opt(b + 1, 3, carry["nat"][3])
                        carry["x"][4] = transpose_opt(b + 1, 4, carry["nat"][4])
                        if cfg.get("carry_q2", False):
                            carry["q"][2] = emit_q(b + 1, 2, carry["x"][2])
                    emit_softmax(a, s_cur,
                                 split=(b == BPC - 1 and
                                        a >= NOPT - 1 - cfg.get("split_a3", 0)))
                    s_cur = s_next
                    # prefetch next item's options while scores stream
                    if b + 1 < BPC:
                        if a == 0:
                            carry["nat"][0] = load_nat(b + 1, 0)
                            carry["nat"][1] = load_nat(b + 1, 1)
                        elif a == 1:
                            carry["nat"][2] = load_nat(b + 1, 2)
                            carry["nat"][3] = load_nat(b + 1, 3)
                        elif a == 2:
                            carry["nat"][4] = load_nat(b + 1, 4)
                    if a == NOPT - 2:
                        # wsum for the last option is complete (it never
                        # scores against itself): overlap its out-matmuls
                        # with the final softmax
                        emit_out_k(NOPT - 1, 0, last=False)

                # ---- tail: AV phased by nn so only 2 out banks live
                # (except on the last item, where latency beats pressure) ----
                _lbl(f"osb b{b}")
                osb = op_.tile([P, SC, H], FP32, tag="osb", name=f"osb_{b}")
                last_item = False
                if last_item:
                    phases = [(0, list(range(NOPT - 1))),
                              (1, list(range(NOPT)))]
                    for nn, ks in phases:
                        for k in ks:
                            emit_out_k(k, nn, last=(k == ks[-1]))
                    for nn, _ks in phases:
                        for mp2 in range(SC):
                            dst = osb[:, mp2, nn * 512 : (nn + 1) * 512]
                            if mp2 == 0:
                                nc.scalar.copy(out=dst, in_=po[(mp2, nn)])
                            else:
                                nc.vector.tensor_copy(out=dst, in_=po[(mp2, nn)])
                            nc.scalar.dma_start(
                                out=out_d.ap()[b].rearrange(
                                    "(sc p) h -> p sc h", p=P)[:, mp2,
                                    nn * 512 : (nn + 1) * 512],
                                in_=osb[:, mp2, nn * 512 : (nn + 1) * 512],
                            )
                else:
                    for nn in range(2):
                        ks = list(range(NOPT - 1)) + ([NOPT - 1] if nn == 1 else [])
                        for k in ks:
                            emit_out_k(k, nn, last=(k == ks[-1]))
                        for mp2 in range(SC):
                            dst = osb[:, mp2, nn * 512 : (nn + 1) * 512]
                            if mp2 == 0:
                                nc.scalar.copy(out=dst, in_=po[(mp2, nn)])
                            else:
                                nc.vector.tensor_copy(out=dst, in_=po[(mp2, nn)])
                            nc.scalar.dma_start(
                                out=out_d.ap()[b].rearrange(
                                    "(sc p) h -> p sc h", p=P)[:, mp2,
                                    nn * 512 : (nn + 1) * 512],
                                in_=osb[:, mp2, nn * 512 : (nn + 1) * 512],
                            )

    nc.compile()
    return nc


def _get_nc(reps: int = 1, cfg: dict | None = None):
    key = f"nc{reps}-{sorted((cfg or {}).items())}"
    if key not in _CACHE:
        _CACHE[key] = _build_bass(reps, cfg)
    return _CACHE[key]


def _in_dtype(cfg: dict | None = None):
    import ml_dtypes

    if (cfg or {}).get("dtype", "bf16") == "bf16":
        return ml_dtypes.bfloat16
    return np.float32


def kernel(**inputs) -> np.ndarray:
    from concourse.bass_utils import run_bass_kernel_spmd

    nc = _get_nc()
    dt = _in_dtype()
    opts = [np.ascontiguousarray(
        np.asarray(inputs[f"option{i + 1}"], dtype=np.float32).astype(dt))
        for i in range(NOPT)]
    W = np.ascontiguousarray(np.asarray(inputs["W"], dtype=np.float32).astype(dt))

    in_maps = []
    for c in range(NCORES):
        m = {f"option{i + 1}": opts[i][c * BPC : (c + 1) * BPC] for i in range(NOPT)}
        m["W"] = W
        in_maps.append(m)

    res = run_bass_kernel_spmd(nc, in_maps, list(range(NCORES)))
    out = np.concatenate([res.results[c]["out"] for c in range(NCORES)], axis=0)
    return np.asarray(out, dtype=np.float32)


# revision 34
# speedup vs baseline: 1.2676x; 1.0493x over previous
"""ChoiceAttention Trainium2 kernel.

Math (per batch item b, per "retain" iteration a over the 5 options):
    q_a = opt_a @ W                              (s, h)
    S_ak[p, r] = q_a[p, :] . opt_k[r, :]         for the 4 options k != a
    w_ak = softmax over k of (S_ak + bias)       (bias cancels: softmax is
                                                  shift-invariant over k)
    out += sum_k w_ak @ opt_k
final out /= 2.

Sharding: data-parallel over batch across 8 NeuronCores (4 items each),
W replicated. No collectives; host concatenates the per-core outputs.

Layout strategy per core / batch item:
    nat_k : opt_k natural layout      (128p, 2 sc, 1024h)  - DMA'd in (bf16)
    x_k   : opt_k transposed (h-major)(128p, 8 hc, 256s)   - PE transposes
    q_a^T : h-major q                 (128p, 8 hc, 256s)   - matmul(W, x_a)
    S_ak^T: scores transposed         (128p, 2 rc, 256p)   - matmul(x_k, q_a^T)
    softmax over the four k tiles elementwise (max-subtract, exp, recip)
    out   : accumulated in 4 PSUM banks over all 40 (a,k,rc) matmul groups
Matmul operands are bf16; scores/softmax accumulate in fp32.

Software pipeline (steady state): item b's tail overlaps the final softmax
and AV matmuls with item b+1's first transposes and q-projections; W is
loaded in 8 per-kc chunks so the first q can chase the DMA.
"""

import numpy as np

B, S, H = 32, 256, 1024
NCORES = 8
BPC = B // NCORES  # batch items per core
P = 128
HC = H // P  # 8 h-chunks
SC = S // P  # 2 s-chunks
NOPT = 5

_CACHE: dict = {}
_label_hook = None


def _lbl(s):
    if _label_hook is not None:
        _label_hook(s)


def _build_bass(reps: int = 1, cfg: dict | None = None):
    cfg = dict(cfg or {})
    BF16_MODE = cfg.get("dtype", "bf16") == "bf16"
    NAT_BUFS = cfg.get("nat_bufs", 10)
    XT_BUFS = cfg.get("xt_bufs", NOPT + 2)
    WS_BUFS = cfg.get("ws_bufs", 5)
    E_BUFS = cfg.get("e_bufs", 5)
    OSB_BUFS = cfg.get("osb_bufs", 2)
    GP_SUB = cfg.get("gp_sub", False)
    PSM = cfg.get("ps_misc", 4)
    PSS = cfg.get("ps_s", 2)
    PSO = cfg.get("ps_o", 2)
    QBUFS = cfg.get("q_bufs", 4)
    SPBUFS = cfg.get("sp_bufs", 8)
    TAIL_SCALAR = cfg.get("tail_scalar", False)
    from contextlib import ExitStack

    import concourse.mybir as mybir
    import concourse.tile as tile
    from concourse import bacc
    from concourse.masks import make_identity

    FP32 = mybir.dt.float32
    F32R = mybir.dt.float32r
    BF16 = mybir.dt.bfloat16
    FP16 = mybir.dt.float16
    DT = BF16 if BF16_MODE else F32R
    AF = mybir.ActivationFunctionType

    nc = bacc.Bacc(debug=bool(cfg.get('debug', False)))

    opt_d = [
        nc.dram_tensor(f"option{i + 1}", (BPC, S, H), DT, kind="ExternalInput")
        for i in range(NOPT)
    ]
    w_d = nc.dram_tensor("W", (H, H), DT, kind="ExternalInput")
    out_d = nc.dram_tensor("out", (BPC, S, H), FP32, kind="ExternalOutput")

    with ExitStack() as ctx:
        tc = ctx.enter_context(tile.TileContext(nc))
        const = ctx.enter_context(tc.tile_pool(name="const", bufs=1))
        natp = ctx.enter_context(tc.tile_pool(name="nat", bufs=NAT_BUFS))
        xp = ctx.enter_context(tc.tile_pool(name="xt", bufs=XT_BUFS))
        qp = ctx.enter_context(tc.tile_pool(name="qq", bufs=QBUFS))
        sp = ctx.enter_context(tc.tile_pool(name="ss", bufs=SPBUFS))
        ep = ctx.enter_context(tc.tile_pool(name="ee", bufs=E_BUFS))
        mp_ = ctx.enter_context(tc.tile_pool(name="mm", bufs=2))
        zp = ctx.enter_context(tc.tile_pool(name="zz", bufs=2))
        rp = ctx.enter_context(tc.tile_pool(name="rr", bufs=2))
        wsp = ctx.enter_context(tc.tile_pool(name="wsum", bufs=WS_BUFS))
        tp = ctx.enter_context(tc.tile_pool(name="tmp", bufs=2))
        op_ = ctx.enter_context(tc.tile_pool(name="osb", bufs=OSB_BUFS))
        ps_misc = ctx.enter_context(tc.tile_pool(name="ps_misc", bufs=PSM, space="PSUM"))
        ps_s = ctx.enter_context(tc.tile_pool(name="ps_s", bufs=PSS, space="PSUM"))
        ps_o = ctx.enter_context(tc.tile_pool(name="ps_o", bufs=PSO, space="PSUM"))

        ident_f = const.tile([P, P], FP32)
        make_identity(nc, ident_f)
        ident = const.tile([P, P], DT)
        nc.vector.tensor_copy(out=ident, in_=ident_f)
        w_all = const.tile([P, HC, H], DT, name="w_all")
        w_sb = [w_all[:, kc, :] for kc in range(HC)]

        from contextlib import nullcontext

        loop_cm = tc.For_i(0, reps, 1) if reps > 1 else nullcontext()
        with loop_cm:
            # cross-batch carried prefetches
            carry = {"nat": {}, "x": {}, "q": {}}

            def load_nat(b, k, split=False):
                _lbl(f"load_nat b{b} k{k}")
                nk = natp.tile([P, SC, H], DT, tag="nat", name=f"nat_{b}_{k}")
                src_ap = opt_d[k].ap()[b].rearrange("(sc p) h -> p sc h", p=P)
                if split:
                    nc.sync.dma_start(out=nk[:, :, : H // 2],
                                      in_=src_ap[:, :, : H // 2])
                    nc.sync.dma_start(out=nk[:, :, H // 2 :],
                                      in_=src_ap[:, :, H // 2 :])
                else:
                    nc.sync.dma_start(out=nk, in_=src_ap)
                return nk

            def transpose_opt(b, k, nk, tail=False):
                # XBAR DMA transpose straight from DRAM: h-major tiles with
                # zero PE/DVE/scalar work (16-bit dtype only)
                _lbl(f"transpose b{b} k{k}")
                xk = xp.tile([P, HC, S], DT, tag="xt", name=f"x_{b}_{k}")
                eng = nc.scalar if (b == 0 or k % 2 == 0) else nc.sync
                eng.dma_start_transpose(out=xk, in_=opt_d[k].ap()[b])
                return xk

            def pe_transpose_opt(b, k, nk):
                # PE-based transpose (b0 head only: fills the PE while W and
                # the XBAR transposes stream in)
                _lbl(f"transpose b{b} k{k}")
                xk = xp.tile([P, HC, S], DT, tag="xt", name=f"x_{b}_{k}")
                for j in range(HC // 2):
                    pt = ps_misc.tile([P, 4, P], DT, tag="ps_misc",
                                      name=f"pt_{b}_{k}_{j}")
                    for d in range(2):
                        hc = 2 * j + d
                        for sc in range(SC):
                            nc.tensor.transpose(
                                out=pt[:, 2 * d + sc, :],
                                in_=nk[:, sc, hc * P : (hc + 1) * P],
                                identity=ident,
                            )
                    dst = xk[:, 2 * j : 2 * j + 2, :]
                    if j % 2 == 0:
                        nc.scalar.copy(out=dst, in_=pt)
                    else:
                        nc.vector.tensor_copy(out=dst, in_=pt)
                return xk

            def emit_q(b, a, xa):
                _lbl(f"q b{b} a{a}")
                qt = qp.tile([P, HC, S], DT, tag="qq", name=f"q_{b}_{a}")
                use_ss = (a < 2) if cfg.get("q_pool", "split") == "split" else False
                pool, ptag = (ps_s, "ps_s") if use_ss else (ps_misc, "ps_misc")
                for half in range(HC // 2):
                    pq = pool.tile([P, 2, S], FP32, tag=ptag,
                                   name=f"pq_{b}_{a}_{half}")
                    for d in range(2):
                        mc = 2 * half + d
                        for kc in range(HC):
                            nc.tensor.matmul(
                                pq[:, d, :],
                                w_sb[kc][:, mc * P : (mc + 1) * P],
                                xa[:, kc, :],
                                start=(kc == 0),
                                stop=(kc == HC - 1),
                            )
                    nc.scalar.copy(out=qt[:, 2 * half : 2 * half + 2, :], in_=pq)
                return qt

            for b in range(BPC):
                # ---- load options (steady state: all carried/prefetched;
                # b==0: deferred below, nat only feeds the AV matmuls) ----
                nat = [carry["nat"].get(k) for k in range(NOPT)]
                if b > 0:
                    nat = [nat[k] or load_nat(b, k) for k in range(NOPT)]

                x = [carry["x"].get(k) for k in range(NOPT)]
                q = [carry["q"].get(a) for a in range(NOPT)]
                carry = {"nat": {}, "x": {}, "q": {}}

                def emit_scores(a):
                    s_sb = []
                    _lbl(f"scores b{b} a{a}")
                    for k in range(NOPT):
                        if k == a:
                            continue
                        st = ps_s.tile([P, SC, S], FP32, tag="ps_s",
                                       name=f"st_{b}_{a}_{k}")
                        for rc in range(SC):
                            for hc in range(HC):
                                nc.tensor.matmul(
                                    st[:, rc, :],
                                    x[k][:, hc, rc * P : (rc + 1) * P],
                                    q[a][:, hc, :],
                                    start=(hc == 0),
                                    stop=(hc == HC - 1),
                                )
                        ssb = sp.tile([P, SC, S], FP16, tag="ss",
                                      name=f"ssb_{b}_{a}_{k}")
                        if len(s_sb) % 2 == 1:
                            nc.vector.tensor_copy(out=ssb, in_=st)
                        else:
                            nc.scalar.copy(out=ssb, in_=st)
                        s_sb.append(ssb)
                    return s_sb

                # wsum[k] accumulates sum_a softmax_weight(a, k): the output
                # matmul collapses to sum_k wsum_k @ opt_k (4x fewer matmuls)
                wsum = [None] * NOPT

                def emit_softmax(a, s_sb, split=False):
                    _lbl(f"softmax b{b} a{a}")
                    MUL = mybir.AluOpType.mult
                    m = mp_.tile([P, SC, S], FP16, tag="mm", name=f"m_{b}_{a}")
                    m2 = mp_.tile([P, SC, S], FP16, tag="m2", name=f"m2_{b}_{a}")
                    e = [ep.tile([P, SC, S], FP16, tag="ee",
                                 name=f"e_{b}_{a}_{k4}") for k4 in range(4)]
                    z = zp.tile([P, SC, S], FP16, tag="zz", name=f"z_{b}_{a}")
                    z23 = zp.tile([P, SC, S], FP16, tag="z2", name=f"z23_{b}_{a}")
                    rcp = rp.tile([P, SC, S], FP16, tag="rr", name=f"r_{b}_{a}")
                    ks = [k for k in range(NOPT) if k != a]
                    newk = [k4 for k4, k in enumerate(ks) if wsum[k] is None]
                    for k in ks:
                        if wsum[k] is None:
                            wsum[k] = wsp.tile([P, SC, S], DT, tag="wsum",
                                               name=f"ws_{b}_{k}")
                    rcs = range(SC) if split else [slice(None)]
                    for rc in rcs:
                        # max over the 4 options: tree split across DVE/Pool
                        nc.vector.tensor_max(m[:, rc], s_sb[0][:, rc],
                                             s_sb[1][:, rc])
                        nc.vector.tensor_max(m2[:, rc], s_sb[2][:, rc],
                                              s_sb[3][:, rc])
                        nc.vector.tensor_max(m[:, rc], m[:, rc], m2[:, rc])
                        for k4 in range(4):
                            sub_eng = nc.gpsimd if (GP_SUB and not split) \
                                else nc.vector
                            sub_eng.tensor_sub(s_sb[k4][:, rc],
                                               s_sb[k4][:, rc], m[:, rc])
                            nc.scalar.activation(out=e[k4][:, rc],
                                                 in_=s_sb[k4][:, rc],
                                                 func=AF.Exp)
                        nc.vector.tensor_add(z[:, rc], e[0][:, rc], e[1][:, rc])
                        nc.vector.tensor_add(z23[:, rc], e[2][:, rc],
                                               e[3][:, rc])
                        nc.vector.tensor_add(z[:, rc], z[:, rc], z23[:, rc])
                        with nc.allow_low_precision(
                                reason="z in [1,4], fp16 recip err ~1e-3"):
                            nc.vector.reciprocal(rcp[:, rc], z[:, rc])
                        for k4, k in enumerate(ks):
                            # (e * 0.5) * rcp folds the final /2 for free
                            if k4 in newk:
                                nc.vector.scalar_tensor_tensor(
                                    wsum[k][:, rc], e[k4][:, rc], 0.5,
                                    rcp[:, rc], MUL, MUL)
                            else:
                                tmp = tp.tile([P, SC, S], DT, tag="tmp",
                                              name=f"t_{b}_{a}_{k4}")
                                nc.vector.scalar_tensor_tensor(
                                    tmp[:, rc], e[k4][:, rc], 0.5,
                                    rcp[:, rc], MUL, MUL)
                                nc.vector.tensor_add(wsum[k][:, rc],
                                                     wsum[k][:, rc],
                                                     tmp[:, rc])

                po = {}
                po_started = {}

                def emit_out_k(k, nn, last):
                    _lbl(f"AV b{b} k{k} nn{nn}")
                    for mp2 in range(SC):
                        key = (mp2, nn)
                        if key not in po:
                            po[key] = ps_o.tile([P, 512], FP32, tag="ps_o",
                                                name=f"po_{b}_{mp2}_{nn}")
                            po_started[key] = False
                        for rc in range(SC):
                            is_last = last and rc == SC - 1
                            nc.tensor.matmul(
                                po[key],
                                wsum[k][:, rc, mp2 * P : (mp2 + 1) * P],
                                nat[k][:, rc, nn * 512 : (nn + 1) * 512],
                                start=(not po_started[key]),
                                stop=is_last,
                            )
                            po_started[key] = True

                # ---- head: fill q pipeline (b==0: interleave with
                # transposes so the first q chases the W-chunk DMAs) ----
                if b == 0:
                    # W as one DMA on the SP ring; nats stream on the ACT
                    # ring feeding PE transposes that fill the W latency
                    nc.sync.dma_start(
                        out=w_all,
                        in_=w_d.ap().rearrange("(kc p) h -> p kc h", p=P))
                    for k in range(NOPT):
                        nat[k] = natp.tile([P, SC, H], DT, tag="nat",
                                           name=f"nat_0_{k}")
                        nc.scalar.dma_start(
                            out=nat[k],
                            in_=opt_d[k].ap()[b].rearrange(
                                "(sc p) h -> p sc h", p=P))
                    for k in range(NOPT):
                        x[k] = pe_transpose_opt(b, k, nat[k])
                for k in range(2):
                    if x[k] is None:
                        x[k] = transpose_opt(b, k, nat[k])
                if q[0] is None:
                    q[0] = emit_q(b, 0, x[0])
                for k in range(2, NOPT):
                    if x[k] is None:
                        x[k] = transpose_opt(b, k, nat[k])
                if q[1] is None:
                    q[1] = emit_q(b, 1, x[1])
                if q[2] is None:
                    q[2] = emit_q(b, 2, x[2])
                s_cur = emit_scores(0)
                for a in range(NOPT):
                    if a + 3 < NOPT:
                        q[a + 3] = emit_q(b, a + 3, x[a + 3])
                    s_next = emit_scores(a + 1) if a + 1 < NOPT else None
                    if a == NOPT - 2 and b + 1 < BPC:
                        # emit next item's transposes + first q-projections
                        # here: their ~22us of PE work hides the latency of
                        # BOTH remaining softmax chains
                        carry["x"][0] = transpose_opt(
                            b + 1, 0, carry["nat"][0], tail=True)
                        carry["x"][1] = transpose_opt(
                            b + 1, 1, carry["nat"][1], tail=True)
                        carry["q"][0] = emit_q(b + 1, 0, carry["x"][0])
                        carry["q"][1] = emit_q(b + 1, 1, carry["x"][1])
                        carry["x"][2] = transpose_opt(b + 1, 2, carry["nat"][2])
                        carry["x"][3] = transpose_opt(b + 1, 3, carry["nat"][3])
                        carry["x"][4] = transpose_opt(b + 1, 4, carry["nat"][4])
                        if cfg.get("carry_q2", False):
                            carry["q"][2] = emit_q(b + 1, 2, carry["x"][2])
                    emit_softmax(a, s_cur,
                                 split=(b == BPC - 1 and
                                        a >= NOPT - 1 - cfg.get("split_a3", 0)))
                    s_cur = s_next
                    # prefetch next item's options while scores stream
                    if b + 1 < BPC:
                        if a == 0:
                            carry["nat"][0] = load_nat(b + 1, 0)
                            carry["nat"][1] = load_nat(b + 1, 1)
                        elif a == 1:
                            carry["nat"][2] = load_nat(b + 1, 2)
                            carry["nat"][3] = load_nat(b + 1, 3)
                        elif a == 2:
                            carry["nat"][4] = load_nat(b + 1, 4)
                    if a == NOPT - 2:
                        # wsum for the last option is complete (it never
                        # scores against itself): overlap its out-matmuls
                        # with the final softmax
                        emit_out_k(NOPT - 1, 0, last=False)

                # ---- tail: AV phased by nn so only 2 out banks live
                # (except on the last item, where latency beats pressure) ----
                _lbl(f"osb b{b}")
                osb = op_.tile([P, SC, H], FP32, tag="osb", name=f"osb_{b}")
                last_item = False
                if last_item:
                    phases = [(0, list(range(NOPT - 1))),
                              (1, list(range(NOPT)))]
                    for nn, ks in phases:
                        for k in ks:
                            emit_out_k(k, nn, last=(k == ks[-1]))
                    for nn, _ks in phases:
                        for mp2 in range(SC):
                            dst = osb[:, mp2, nn * 512 : (nn + 1) * 512]
                            if mp2 == 0:
                                nc.scalar.copy(out=dst, in_=po[(mp2, nn)])
                            else:
                                nc.vector.tensor_copy(out=dst, in_=po[(mp2, nn)])
                            nc.scalar.dma_start(
                                out=out_d.ap()[b].rearrange(
                                    "(sc p) h -> p sc h", p=P)[:, mp2,
                                    nn * 512 : (nn + 1) * 512],
                                in_=osb[:, mp2, nn * 512 : (nn + 1) * 512],
                            )
                else:
                    for nn in range(2):
                        ks = list(range(NOPT - 1)) + ([NOPT - 1] if nn == 1 else [])
                        for k in ks:
                            emit_out_k(k, nn, last=(k == ks[-1]))
                        for mp2 in range(SC):
                            dst = osb[:, mp2, nn * 512 : (nn + 1) * 512]
                            if mp2 == 0:
                                nc.scalar.copy(out=dst, in_=po[(mp2, nn)])
                            else:
                                nc.vector.tensor_copy(out=dst, in_=po[(mp2, nn)])
                            nc.scalar.dma_start(
                                out=out_d.ap()[b].rearrange(
                                    "(sc p) h -> p sc h", p=P)[:, mp2,
                                    nn * 512 : (nn + 1) * 512],
                                in_=osb[:, mp2, nn * 512 : (nn + 1) * 512],
                            )

    nc.compile()
    return nc


def _get_nc(reps: int = 1, cfg: dict | None = None):
    key = f"nc{reps}-{sorted((cfg or {}).items())}"
    if key not in _CACHE:
        _CACHE[key] = _build_bass(reps, cfg)
    return _CACHE[key]


def _in_dtype(cfg: dict | None = None):
    import ml_dtypes

    if (cfg or {}).get("dtype", "bf16") == "bf16":
        return ml_dtypes.bfloat16
    return np.float32


def kernel(**inputs) -> np.ndarray:
    from concourse.bass_utils import run_bass_kernel_spmd

    nc = _get_nc()
    dt = _in_dtype()
    opts = [np.ascontiguousarray(
        np.asarray(inputs[f"option{i + 1}"], dtype=np.float32).astype(dt))
        for i in range(NOPT)]
    W = np.ascontiguousarray(np.asarray(inputs["W"], dtype=np.float32).astype(dt))

    in_maps = []
    for c in range(NCORES):
        m = {f"option{i + 1}": opts[i][c * BPC : (c + 1) * BPC] for i in range(NOPT)}
        m["W"] = W
        in_maps.append(m)

    res = run_bass_kernel_spmd(nc, in_maps, list(range(NCORES)))
    out = np.concatenate([res.results[c]["out"] for c in range(NCORES)], axis=0)
    return np.asarray(out, dtype=np.float32)
